# revision 10
# baseline (speedup 1.0000x reference)
"""Circular-convolution helper kernel for Trainium2 (8 NeuronCores).

Math: out[i] = sum_b sum_t x1[b,(i-t)%D] * x2[b,t]
            = sum_j G[j, (i-j)%D]   where G = x1^T @ x2  ([D, D], K=B contraction)

Sharding: G's rows are split across the 8 cores (core c owns rows
[128c, 128c+128)).  Per core, column-chunked and pipelined:
  1. A = x1c^T @ x2 into PSUM (K=128 matmul, 4 column chunks, one PSUM bank
     each so matmuls never serialize against the copies)
  2. PSUM -> SBUF copies on the Vector engine
  3. scatter A into a DRAM buffer gd with row pitch 2D so the circular
     anti-diagonals become rows: gd[m, k] = A[m, k % D], k in [896, 2048)
     (write triggers on the Scalar queue; only the columns the diagonal
     read touches are written)
  4. diagonal read H[m, i] = gd[m, D + i - m] = A[m, (i-m) % D]
     (row stride 2D-1 in the flat DRAM view; triggers on the Sync queue)
  5. ones-matmul partition collapse: part[i] = sum_m H[m, i]
Host rotates each core's partial by 128c and sums.

Chunk order 3,0,1,2 puts the wrap-around tail (A cols [896,1024), needed by
the first diagonal-read chunk) in DRAM early so reads stream behind writes.
"""

import numpy as np

B = 128
DIM = 1024
NCORES = 8
CHUNK = DIM // NCORES  # 128 rows of G per core
NHALF = 512
NCHUNKS = 4
CW = DIM // NCHUNKS  # 256

USE_F32R = False  # single-pass fp32r matmuls (reduced precision streaming)


_cached = {}


def _build():
    key = ("nc", USE_F32R)
    if key in _cached:
        return _cached[key]

    import concourse.bass as bass
    import concourse.mybir as mybir
    from concourse import bacc
    from concourse.tile import TileContext

    f32 = mybir.dt.float32

    nc = bacc.Bacc("TRN2", target_bir_lowering=False, debug=False)

    x1c = nc.dram_tensor("x1c", [B, CHUNK], f32, kind="ExternalInput")
    x2 = nc.dram_tensor("x2", [B, DIM], f32, kind="ExternalInput")
    out = nc.dram_tensor("out", [1, DIM], f32, kind="ExternalOutput")
    # diag scratch: row pitch 2D, only cols [896, 2048) ever written/read
    gd = nc.dram_tensor("gd", [CHUNK, 2 * DIM], f32, kind="Internal")

    with TileContext(nc) as tc:
        with (
            tc.tile_pool(name="sb", bufs=1) as sb,
            tc.tile_pool(name="ps", bufs=1, space="PSUM") as ps,
        ):
            order = [3, 0, 1, 2]

            # loads: x1c, then x2 as two halves (high half first — chunk 3
            # is consumed first)
            x1t = sb.tile([B, CHUNK], f32)
            nc.sync.dma_start(x1t[:], x1c.ap())
            x2t = sb.tile([B, DIM], f32)
            x2ap = x2.ap()
            nc.sync.dma_start(x2t[:, NHALF:DIM], x2ap[:, NHALF:DIM])
            nc.sync.dma_start(x2t[:, 0:NHALF], x2ap[:, 0:NHALF])

            gs = [
                ps.tile([CHUNK, CW], f32, name=f"g{i}", tag=f"g{i}")
                for i in range(NCHUNKS)
            ]
            a = sb.tile([CHUNK, DIM], f32)
            ht = sb.tile([CHUNK, DIM], f32)
            ones = sb.tile([CHUNK, 1], f32)
            nc.vector.memset(ones[:], 1.0)
            o = ps.tile([1, DIM], f32)
            gd_ap = gd.ap()

            if USE_F32R:
                f32r = mybir.dt.float32r
                x1_mm = x1t[:].bitcast(f32r)
                x2_mm = x2t[:].bitcast(f32r)
                ones_mm = ones[:].bitcast(f32r)
                ht_mm = ht[:].bitcast(f32r)
            else:
                x1_mm = x1t[:]
                x2_mm = x2t[:]
                ones_mm = ones[:]
                ht_mm = ht[:]

            for ch in order:
                lo, hi = ch * CW, (ch + 1) * CW
                nc.tensor.matmul(
                    gs[ch][:], x1_mm, x2_mm[:, lo:hi], start=True, stop=True
                )
                nc.vector.tensor_copy(a[:, lo:hi], gs[ch][:])
                # scatter trigger on the Scalar HWDGE queue
                nc.scalar.dma_start(gd_ap[:, DIM + lo : DIM + hi], a[:, lo:hi])
                if ch == 3:
                    # wrap tail: gd[:, 896:1024] = A[:, 896:1024]
                    nc.scalar.dma_start(gd_ap[:, 896:DIM], a[:, 896:DIM])

            # diagonal reads (Sync queue) + ones-matmul collapse, streamed
            for ch in range(NCHUNKS):
                lo, hi = ch * CW, (ch + 1) * CW
                diag = bass.AP(gd, DIM + lo, [[2 * DIM - 1, CHUNK], [1, CW]])
                nc.sync.dma_start(ht[:, lo:hi], diag)
                nc.tensor.matmul(
                    o[:, lo:hi], ones_mm, ht_mm[:, lo:hi], start=True, stop=True
                )

            ot = sb.tile([1, DIM], f32)
            nc.scalar.copy(ot[:, 0:NHALF], o[:, 0:NHALF])
            nc.vector.tensor_copy(ot[:, NHALF:DIM], o[:, NHALF:DIM])
            nc.sync.dma_start(out.ap(), ot[:])

    nc.compile()
    _cached[key] = nc
    return nc


def _in_maps(input1, input2):
    x1 = np.ascontiguousarray(np.asarray(input1, dtype=np.float32))
    x2 = np.ascontiguousarray(np.asarray(input2, dtype=np.float32))
    return [
        {
            "x1c": np.ascontiguousarray(x1[:, c * CHUNK : (c + 1) * CHUNK]),
            "x2": x2,
        }
        for c in range(NCORES)
    ]


def _combine(results):
    total = np.zeros(DIM, np.float64)
    for c in range(NCORES):
        total += np.roll(results[c]["out"][0].astype(np.float64), CHUNK * c)
    return total.astype(np.float32).reshape(1, 1, DIM)


def _run(input1, input2, **kwargs):
    from concourse import bass_utils

    nc = _build()
    res = bass_utils.run_bass_kernel_spmd(
        nc, _in_maps(input1, input2), core_ids=list(range(NCORES)), **kwargs
    )
    return res


def kernel(input1, input2):
    res = _run(input1, input2)
    return _combine(res.results)


# revision 11
# speedup vs baseline: 1.0341x; 1.0341x over previous
"""Circular-convolution helper kernel for Trainium2 (8 NeuronCores).

Math: out[i] = sum_b sum_t x1[b,(i-t)%D] * x2[b,t]
            = sum_j G[j, (i-j)%D]   where G = x1^T @ x2  ([D, D], K=B contraction)

Sharding: G's rows are split across the 8 cores (core c owns rows
[128c, 128c+128)).  Per core:
  1. A = x1c^T @ x2 into PSUM (K=128 matmul, 4 column chunks into separate
     PSUM banks so matmuls never serialize against the copies)
  2. PSUM -> SBUF copies on the Vector engine
  3. scatter A into DRAM gd with row pitch 2D: gd[m, k] = A[m, k % D] for
     k in [896, 2048).  Row-split x4 (4 KiB segments, parallel HW queues);
     the wrap tail (cols [896,1024)) fires right after the first copy.
  4. diagonal read H[m, i] = gd[m, D + i - m] = A[m, (i-m) % D]: flat row
     stride 2D-1, row-split x4 so segments stay 4 KiB.
  5. ones-matmul partition collapse: part[i] = sum_m H[m, i]
Host rotates each core's partial by 128c and sums.

DMA triggers cost ~600 ns of sequencer time each and a queue moves ~4 KiB
per ~32 ns, so transfers are split across the Sync and Scalar (Activation)
HWDGE queues with the largest segments the layout allows.
"""

import numpy as np

B = 128
DIM = 1024
NCORES = 8
CHUNK = DIM // NCORES  # 128 rows of G per core
NHALF = 512
NCHUNKS = 4
CW = DIM // NCHUNKS  # 256
RSPLIT = 4  # row-split factor for big DMAs
RW = CHUNK // RSPLIT  # 32 rows per DMA

USE_F32R = False  # single-pass fp32r matmuls (reduced precision streaming)


_cached = {}


def _build():
    key = ("nc", USE_F32R)
    if key in _cached:
        return _cached[key]

    import concourse.bass as bass
    import concourse.mybir as mybir
    from concourse import bacc
    from concourse.tile import TileContext

    f32 = mybir.dt.float32

    nc = bacc.Bacc("TRN2", target_bir_lowering=False, debug=False)

    x1c = nc.dram_tensor("x1c", [B, CHUNK], f32, kind="ExternalInput")
    x2 = nc.dram_tensor("x2", [B, DIM], f32, kind="ExternalInput")
    out = nc.dram_tensor("out", [1, DIM], f32, kind="ExternalOutput")
    # diag scratch: row pitch 2D, only cols [896, 2048) ever written/read
    gd = nc.dram_tensor("gd", [CHUNK, 2 * DIM], f32, kind="Internal")

    with TileContext(nc) as tc:
        with (
            tc.tile_pool(name="sb", bufs=1) as sb,
            tc.tile_pool(name="ps", bufs=1, space="PSUM") as ps,
        ):
            order = [3, 0, 1, 2]

            # loads, row-split: x2 on Sync, x1c on Scalar
            x1t = sb.tile([B, CHUNK], f32)
            x2t = sb.tile([B, DIM], f32)
            x1ap = x1c.ap()
            x2ap = x2.ap()
            for q in range(RSPLIT):
                r0, r1 = q * RW, (q + 1) * RW
                nc.sync.dma_start(x2t[r0:r1, :], x2ap[r0:r1, :])
                nc.scalar.dma_start(x1t[r0:r1, :], x1ap[r0:r1, :])

            gs = [
                ps.tile([CHUNK, CW], f32, name=f"g{i}", tag=f"g{i}")
                for i in range(NCHUNKS)
            ]
            a = sb.tile([CHUNK, DIM], f32)
            ht = sb.tile([CHUNK, DIM], f32)
            ones = sb.tile([CHUNK, 1], f32)
            nc.vector.memset(ones[:], 1.0)
            o = ps.tile([1, DIM], f32)
            gd_ap = gd.ap()

            if USE_F32R:
                f32r = mybir.dt.float32r
                x1_mm = x1t[:].bitcast(f32r)
                x2_mm = x2t[:].bitcast(f32r)
                ones_mm = ones[:].bitcast(f32r)
                ht_mm = ht[:].bitcast(f32r)
            else:
                x1_mm = x1t[:]
                x2_mm = x2t[:]
                ones_mm = ones[:]
                ht_mm = ht[:]

            for ch in order:
                lo, hi = ch * CW, (ch + 1) * CW
                nc.tensor.matmul(
                    gs[ch][:], x1_mm, x2_mm[:, lo:hi], start=True, stop=True
                )
                nc.vector.tensor_copy(a[:, lo:hi], gs[ch][:])
                if ch == 3:
                    # wrap tail: gd[:, 896:1024] = A[:, 896:1024], row-split x2
                    for q in range(2):
                        r0, r1 = q * 64, (q + 1) * 64
                        nc.scalar.dma_start(
                            gd_ap[r0:r1, 896:DIM], a[r0:r1, 896:DIM]
                        )

            # main scatter: gd[:, D:2D] = A, row-split x4 across both queues
            for q in range(RSPLIT):
                r0, r1 = q * RW, (q + 1) * RW
                eng = nc.sync if q % 2 == 0 else nc.scalar
                eng.dma_start(gd_ap[r0:r1, DIM : 2 * DIM], a[r0:r1, :])

            # diagonal read, full width, row-split x4:
            # H[m, i] = gd_flat[D + m*(2D-1) + i]
            for q in range(RSPLIT):
                r0 = q * RW
                diag = bass.AP(
                    gd, DIM + r0 * (2 * DIM - 1), [[2 * DIM - 1, RW], [1, DIM]]
                )
                eng = nc.sync if q % 2 == 0 else nc.scalar
                eng.dma_start(ht[r0 : r0 + RW, :], diag)

            for ch in range(NCHUNKS):
                lo, hi = ch * CW, (ch + 1) * CW
                nc.tensor.matmul(
                    o[:, lo:hi], ones_mm, ht_mm[:, lo:hi], start=True, stop=True
                )

            ot = sb.tile([1, DIM], f32)
            nc.vector.tensor_copy(ot[:, 0:NHALF], o[:, 0:NHALF])
            nc.vector.tensor_copy(ot[:, NHALF:DIM], o[:, NHALF:DIM])
            nc.sync.dma_start(out.ap(), ot[:])

    nc.compile()
    _cached[key] = nc
    return nc


def _in_maps(input1, input2):
    x1 = np.ascontiguousarray(np.asarray(input1, dtype=np.float32))
    x2 = np.ascontiguousarray(np.asarray(input2, dtype=np.float32))
    return [
        {
            "x1c": np.ascontiguousarray(x1[:, c * CHUNK : (c + 1) * CHUNK]),
            "x2": x2,
        }
        for c in range(NCORES)
    ]


def _combine(results):
    total = np.zeros(DIM, np.float64)
    for c in range(NCORES):
        total += np.roll(results[c]["out"][0].astype(np.float64), CHUNK * c)
    return total.astype(np.float32).reshape(1, 1, DIM)


def _run(input1, input2, **kwargs):
    from concourse import bass_utils

    nc = _build()
    res = bass_utils.run_bass_kernel_spmd(
        nc, _in_maps(input1, input2), core_ids=list(range(NCORES)), **kwargs
    )
    return res


def kernel(input1, input2):
    res = _run(input1, input2)
    return _combine(res.results)


# revision 14
# speedup vs baseline: 1.1166x; 1.0798x over previous
"""Circular-convolution helper kernel for Trainium2 (8 NeuronCores).

Math: out[i] = sum_b sum_t x1[b,(i-t)%D] * x2[b,t]
            = sum_j G[j, (i-j)%D]   where G = x1^T @ x2  ([D, D], K=B contraction)

Sharding: G's rows are split across the 8 cores (core c owns rows
[128c, 128c+128)).  Per core:
  1. A = x1c^T @ x2 into PSUM (K=128 matmul, 4 column chunks into separate
     PSUM banks so matmuls never serialize against the copies)
  2. PSUM -> SBUF copies on the Vector engine
  3. scatter A into DRAM gd with row pitch 2D: gd[m, k] = A[m, k % D] for
     k in [896, 2048), row-split so 4 KiB segments spread over both HWDGE
     queues (Sync + Activation)
  4. diagonal read H[m, i] = gd[m, D + i - m] = A[m, (i-m) % D]: flat row
     stride 2D-1, row-split x4
  5. ones-matmul partition collapse: part[i] = sum_m H[m, i]
Host rotates each core's partial by 128c and sums.

USE_F32R runs every matmul in fp32r (single-pass PE streaming, 4x faster
than fp32's LOW/HIGH passes).  The whole data path is declared fp32r and the
host pre-truncates input mantissas so every producer hands the PE values
already in fp32r form (walrus' BIR verifier requires this).
"""

import numpy as np

B = 128
DIM = 1024
NCORES = 8
CHUNK = DIM // NCORES  # 128 rows of G per core
NHALF = 512
NCHUNKS = 4
CW = DIM // NCHUNKS  # 256
RSPLIT = 4  # row-split factor for big DMAs
RW = CHUNK // RSPLIT  # 32 rows per DMA

USE_F32R = True
F32R_CHOP_BITS = 13  # keep 10 mantissa bits (tf32-like)


_cached = {}


def _build():
    key = ("nc", USE_F32R)
    if key in _cached:
        return _cached[key]

    import concourse.bass as bass
    import concourse.mybir as mybir
    from concourse import bacc
    from concourse.tile import TileContext

    f32 = mybir.dt.float32
    dt_mm = mybir.dt.float32r if USE_F32R else f32

    nc = bacc.Bacc("TRN2", target_bir_lowering=False, debug=False)

    x1c = nc.dram_tensor("x1c", [B, CHUNK], dt_mm, kind="ExternalInput")
    x2 = nc.dram_tensor("x2", [B, DIM], dt_mm, kind="ExternalInput")
    out = nc.dram_tensor("out", [1, DIM], f32, kind="ExternalOutput")
    # diag scratch: row pitch 2D, only cols [896, 2048) ever written/read
    gd = nc.dram_tensor("gd", [CHUNK, 2 * DIM], dt_mm, kind="Internal")

    with TileContext(nc) as tc:
        with (
            tc.tile_pool(name="sb", bufs=1) as sb,
            tc.tile_pool(name="ps", bufs=1, space="PSUM") as ps,
        ):
            order = [3, 0, 1, 2]

            # loads, row-split across both HWDGE queues
            x1t = sb.tile([B, CHUNK], dt_mm)
            x2t = sb.tile([B, DIM], dt_mm)
            x1ap = x1c.ap()
            x2ap = x2.ap()
            for q in range(RSPLIT):
                r0, r1 = q * RW, (q + 1) * RW
                e_x2 = nc.sync if q % 2 == 0 else nc.scalar
                e_x1 = nc.scalar if q % 2 == 0 else nc.sync
                e_x2.dma_start(x2t[r0:r1, :], x2ap[r0:r1, :])
                e_x1.dma_start(x1t[r0:r1, :], x1ap[r0:r1, :])

            gs = [
                ps.tile([CHUNK, CW], f32, name=f"g{i}", tag=f"g{i}")
                for i in range(NCHUNKS)
            ]
            a = sb.tile([CHUNK, DIM], dt_mm)
            ht = sb.tile([CHUNK, DIM], dt_mm)
            ones = sb.tile([CHUNK, 1], dt_mm)
            if USE_F32R:
                ones_f = sb.tile([CHUNK, 1], f32)
                nc.vector.memset(ones_f[:], 1.0)
                nc.vector.tensor_copy(ones[:], ones_f[:])
            else:
                nc.vector.memset(ones[:], 1.0)
            o = ps.tile([1, DIM], f32)
            gd_ap = gd.ap()

            for ch in order:
                lo, hi = ch * CW, (ch + 1) * CW
                nc.tensor.matmul(
                    gs[ch][:], x1t[:], x2t[:, lo:hi], start=True, stop=True
                )
                # PSUM (f32) -> SBUF; DVE rounds to fp32r when enabled
                nc.vector.tensor_copy(a[:, lo:hi], gs[ch][:])
                if ch == 3:
                    # wrap tail: gd[:, 896:1024] = A[:, 896:1024], row-split
                    for q in range(RSPLIT):
                        r0, r1 = q * RW, (q + 1) * RW
                        eng = nc.scalar if q % 2 == 0 else nc.sync
                        eng.dma_start(gd_ap[r0:r1, 896:DIM], a[r0:r1, 896:DIM])

            # main scatter: gd[:, D:2D] = A, row-split x4 across both queues
            for q in range(RSPLIT):
                r0, r1 = q * RW, (q + 1) * RW
                eng = nc.sync if q % 2 == 0 else nc.scalar
                eng.dma_start(gd_ap[r0:r1, DIM : 2 * DIM], a[r0:r1, :])

            # diagonal read, full width, row-split x4:
            # H[m, i] = gd_flat[D + m*(2D-1) + i]
            for q in range(RSPLIT):
                r0 = q * RW
                diag = bass.AP(
                    gd, DIM + r0 * (2 * DIM - 1), [[2 * DIM - 1, RW], [1, DIM]]
                )
                eng = nc.sync if q % 2 == 0 else nc.scalar
                eng.dma_start(ht[r0 : r0 + RW, :], diag)

            ot = sb.tile([1, DIM], f32)
            for ch in range(NCHUNKS):
                lo, hi = ch * CW, (ch + 1) * CW
                nc.tensor.matmul(
                    o[:, lo:hi], ones[:], ht[:, lo:hi], start=True, stop=True
                )
                nc.vector.tensor_copy(ot[:, lo:hi], o[:, lo:hi])
            nc.sync.dma_start(out.ap(), ot[:])

    nc.compile()
    _cached[key] = nc
    return nc


def _chop(x):
    if not USE_F32R:
        return x
    u = x.view(np.uint32) & np.uint32((0xFFFFFFFF << F32R_CHOP_BITS) & 0xFFFFFFFF)
    return u.view(np.float32)


def _in_maps(input1, input2):
    x1 = np.ascontiguousarray(np.asarray(input1, dtype=np.float32))
    x2 = _chop(np.ascontiguousarray(np.asarray(input2, dtype=np.float32)))
    return [
        {
            "x1c": _chop(
                np.ascontiguousarray(x1[:, c * CHUNK : (c + 1) * CHUNK])
            ),
            "x2": x2,
        }
        for c in range(NCORES)
    ]


def _combine(results):
    total = np.zeros(DIM, np.float64)
    for c in range(NCORES):
        total += np.roll(results[c]["out"][0].astype(np.float64), CHUNK * c)
    return total.astype(np.float32).reshape(1, 1, DIM)


def _run(input1, input2, **kwargs):
    from concourse import bass_utils

    nc = _build()
    res = bass_utils.run_bass_kernel_spmd(
        nc, _in_maps(input1, input2), core_ids=list(range(NCORES)), **kwargs
    )
    return res


def kernel(input1, input2):
    res = _run(input1, input2)
    return _combine(res.results)


# revision 15
# speedup vs baseline: 1.1476x; 1.0278x over previous
"""Circular-convolution helper kernel for Trainium2 (8 NeuronCores).

Math: out[i] = sum_b sum_t x1[b,(i-t)%D] * x2[b,t]
            = sum_j G[j, (i-j)%D]   where G = x1^T @ x2  ([D, D], K=B contraction)

Sharding: G's rows are split across the 8 cores (core c owns rows
[128c, 128c+128)).  Per core:
  1. A = x1c^T @ x2 into PSUM (K=128 matmul, 4 column chunks into separate
     PSUM banks so matmuls never serialize against the copies)
  2. PSUM -> SBUF copies on the Vector engine
  3. scatter A into DRAM gd with row pitch 2D: gd[m, k] = A[m, k % D] for
     k in [896, 2048), row-split so 4 KiB segments spread over both HWDGE
     queues (Sync + Activation)
  4. diagonal read H[m, i] = gd[m, D + i - m] = A[m, (i-m) % D]: flat row
     stride 2D-1, row-split x4
  5. ones-matmul partition collapse: part[i] = sum_m H[m, i]
Host rotates each core's partial by 128c and sums.

USE_F32R runs every matmul in fp32r (single-pass PE streaming, 4x faster
than fp32's LOW/HIGH passes).  The whole data path is declared fp32r and the
host pre-truncates input mantissas so every producer hands the PE values
already in fp32r form (walrus' BIR verifier requires this).
"""

import numpy as np

B = 128
DIM = 1024
NCORES = 8
CHUNK = DIM // NCORES  # 128 rows of G per core
NHALF = 512
NCHUNKS = 4
CW = DIM // NCHUNKS  # 256
RSPLIT = 4  # row-split factor for big DMAs
RW = CHUNK // RSPLIT  # 32 rows per DMA

USE_F32R = True
F32R_CHOP_BITS = 13  # keep 10 mantissa bits (tf32-like)


_cached = {}


def _build():
    key = ("nc", USE_F32R)
    if key in _cached:
        return _cached[key]

    import concourse.bass as bass
    import concourse.mybir as mybir
    from concourse import bacc
    from concourse.tile import TileContext

    f32 = mybir.dt.float32
    dt_mm = mybir.dt.float32r if USE_F32R else f32

    nc = bacc.Bacc("TRN2", target_bir_lowering=False, debug=False)

    x1c = nc.dram_tensor("x1c", [B, CHUNK], dt_mm, kind="ExternalInput")
    x2 = nc.dram_tensor("x2", [B, DIM], dt_mm, kind="ExternalInput")
    out = nc.dram_tensor("out", [1, DIM], f32, kind="ExternalOutput")
    # diag scratch: row pitch 2D, only cols [896, 2048) ever written/read
    gd = nc.dram_tensor("gd", [CHUNK, 2 * DIM], dt_mm, kind="Internal")

    with TileContext(nc) as tc:
        with (
            tc.tile_pool(name="sb", bufs=1) as sb,
            tc.tile_pool(name="ps", bufs=1, space="PSUM") as ps,
        ):
            order = [3, 0, 1, 2]

            # loads, row-split across both HWDGE queues
            x1t = sb.tile([B, CHUNK], dt_mm)
            x2t = sb.tile([B, DIM], dt_mm)
            x1ap = x1c.ap()
            x2ap = x2.ap()
            for q in range(RSPLIT):
                r0, r1 = q * RW, (q + 1) * RW
                e_x2 = nc.sync if q % 2 == 0 else nc.scalar
                e_x1 = nc.scalar if q % 2 == 0 else nc.sync
                e_x2.dma_start(x2t[r0:r1, :], x2ap[r0:r1, :])
                e_x1.dma_start(x1t[r0:r1, :], x1ap[r0:r1, :])

            gs = [
                ps.tile([CHUNK, CW], f32, name=f"g{i}", tag=f"g{i}")
                for i in range(NCHUNKS)
            ]
            a = sb.tile([CHUNK, DIM], dt_mm)
            ht = sb.tile([CHUNK, DIM], dt_mm)
            ones = sb.tile([CHUNK, 1], dt_mm)
            if USE_F32R:
                ones_f = sb.tile([CHUNK, 1], f32)
                nc.vector.memset(ones_f[:], 1.0)
                nc.vector.tensor_copy(ones[:], ones_f[:])
            else:
                nc.vector.memset(ones[:], 1.0)
            o = ps.tile([1, DIM], f32)
            gd_ap = gd.ap()

            for ch in order:
                lo, hi = ch * CW, (ch + 1) * CW
                nc.tensor.matmul(
                    gs[ch][:], x1t[:], x2t[:, lo:hi], start=True, stop=True
                )
                # PSUM (f32) -> SBUF; DVE rounds to fp32r when enabled
                nc.vector.tensor_copy(a[:, lo:hi], gs[ch][:])
                if ch == 3:
                    # wrap tail: gd[:, 896:1024] = A[:, 896:1024] (SWDGE —
                    # software descriptor gen, off the shared HW-DGE unit)
                    nc.gpsimd.dma_start(gd_ap[:, 896:DIM], a[:, 896:DIM])

            # main scatter: gd[:, D:2D] = A on SWDGE, split x2
            for q in range(2):
                r0, r1 = q * 64, (q + 1) * 64
                nc.gpsimd.dma_start(gd_ap[r0:r1, DIM : 2 * DIM], a[r0:r1, :])

            # diagonal read, full width, row-split x4:
            # H[m, i] = gd_flat[D + m*(2D-1) + i]
            for q in range(RSPLIT):
                r0 = q * RW
                diag = bass.AP(
                    gd, DIM + r0 * (2 * DIM - 1), [[2 * DIM - 1, RW], [1, DIM]]
                )
                eng = nc.sync if q % 2 == 0 else nc.scalar
                eng.dma_start(ht[r0 : r0 + RW, :], diag)

            ot = sb.tile([1, DIM], f32)
            for ch in range(NCHUNKS):
                lo, hi = ch * CW, (ch + 1) * CW
                nc.tensor.matmul(
                    o[:, lo:hi], ones[:], ht[:, lo:hi], start=True, stop=True
                )
                nc.vector.tensor_copy(ot[:, lo:hi], o[:, lo:hi])
            nc.sync.dma_start(out.ap(), ot[:])

    nc.compile()
    _cached[key] = nc
    return nc


def _chop(x):
    if not USE_F32R:
        return x
    u = x.view(np.uint32) & np.uint32((0xFFFFFFFF << F32R_CHOP_BITS) & 0xFFFFFFFF)
    return u.view(np.float32)


def _in_maps(input1, input2):
    x1 = np.ascontiguousarray(np.asarray(input1, dtype=np.float32))
    x2 = _chop(np.ascontiguousarray(np.asarray(input2, dtype=np.float32)))
    return [
        {
            "x1c": _chop(
                np.ascontiguousarray(x1[:, c * CHUNK : (c + 1) * CHUNK])
            ),
            "x2": x2,
        }
        for c in range(NCORES)
    ]


def _combine(results):
    total = np.zeros(DIM, np.float64)
    for c in range(NCORES):
        total += np.roll(results[c]["out"][0].astype(np.float64), CHUNK * c)
    return total.astype(np.float32).reshape(1, 1, DIM)


def _run(input1, input2, **kwargs):
    from concourse import bass_utils

    nc = _build()
    res = bass_utils.run_bass_kernel_spmd(
        nc, _in_maps(input1, input2), core_ids=list(range(NCORES)), **kwargs
    )
    return res


def kernel(input1, input2):
    res = _run(input1, input2)
    return _combine(res.results)


# revision 16
# speedup vs baseline: 1.1812x; 1.0292x over previous
"""Circular-convolution helper kernel for Trainium2 (8 NeuronCores).

Math: out[i] = sum_b sum_t x1[b,(i-t)%D] * x2[b,t]
            = sum_j G[j, (i-j)%D]   where G = x1^T @ x2  ([D, D], K=B contraction)

Sharding: G's rows are split across the 8 cores (core c owns rows
[128c, 128c+128)).  Per core:
  1. load xin = [x2 | x1c] as one [128, D+128] tensor (4.5 KiB rows keep the
     DMA descriptor count minimal), row-split across the two HWDGE queues
  2. A = x1c^T @ x2 into PSUM (K=128 fp32r matmul, 4 column chunks into
     separate PSUM banks)
  3. PSUM -> SBUF casts (fp32 -> fp32r) into a [128, 128+D] staging tile
     laid out as [A[:, 896:1024] | A] so the DRAM scatter is ONE contiguous
     region per row
  4. scatter to gd[128, 1152]: flat[1152 m + p] = staged row (4.5 KiB rows)
  5. diagonal read H[m, i] = A[m, (i-m) % D] = gd_flat[128 + 1151 m + i]
  6. ones-matmul partition collapse (fp32r): part[i] = sum_m H[m, i]
Host rotates each core's partial by 128c and sums.

Everything on the PE runs in fp32r (single-pass streaming): the host
pre-truncates input mantissas to 10 bits so every producer hands the PE
values already in fp32r form (walrus' BIR verifier requires this).
"""

import numpy as np

B = 128
DIM = 1024
NCORES = 8
CHUNK = DIM // NCORES  # 128 rows of G per core
NHALF = 512
NCHUNKS = 4
CW = DIM // NCHUNKS  # 256
XW = DIM + CHUNK  # packed input width: x2 | x1c
AW = CHUNK + DIM  # staging width: wrap tail | A

USE_F32R = True
F32R_CHOP_BITS = 13  # keep 10 mantissa bits (tf32-like)


_cached = {}


def _build():
    key = ("nc", USE_F32R)
    if key in _cached:
        return _cached[key]

    import concourse.bass as bass
    import concourse.mybir as mybir
    from concourse import bacc
    from concourse.tile import TileContext

    f32 = mybir.dt.float32
    dt_mm = mybir.dt.float32r if USE_F32R else f32

    nc = bacc.Bacc("TRN2", target_bir_lowering=False, debug=False)

    xin = nc.dram_tensor("xin", [B, XW], dt_mm, kind="ExternalInput")
    out = nc.dram_tensor("out", [1, DIM], f32, kind="ExternalOutput")
    gd = nc.dram_tensor("gd", [CHUNK, AW], dt_mm, kind="Internal")

    with TileContext(nc) as tc:
        with (
            tc.tile_pool(name="sb", bufs=1) as sb,
            tc.tile_pool(name="ps", bufs=1, space="PSUM") as ps,
        ):
            xt = sb.tile([B, XW], dt_mm)
            xin_ap = xin.ap()
            nc.sync.dma_start(xt[0:64, :], xin_ap[0:64, :])
            nc.scalar.dma_start(xt[64:B, :], xin_ap[64:B, :])
            x1_mm = xt[:, DIM:XW]

            gs = [
                ps.tile([CHUNK, CW], f32, name=f"g{i}", tag=f"g{i}")
                for i in range(NCHUNKS)
            ]
            a = sb.tile([CHUNK, AW], dt_mm)
            ht = sb.tile([CHUNK, DIM], dt_mm)
            ones = sb.tile([CHUNK, 1], dt_mm)
            if USE_F32R:
                ones_f = sb.tile([CHUNK, 1], f32)
                nc.vector.memset(ones_f[:], 1.0)
                nc.vector.tensor_copy(ones[:], ones_f[:])
            else:
                nc.vector.memset(ones[:], 1.0)
            o = ps.tile([1, DIM], f32)
            gd_ap = gd.ap()

            # A chunks; staging layout: a[:, 0:128] = A[:, 896:1024] (wrap
            # tail), a[:, 128:1152] = A[:, 0:1024]
            order = [3, 0, 1, 2]
            for ch in order:
                lo, hi = ch * CW, (ch + 1) * CW
                nc.tensor.matmul(
                    gs[ch][:], x1_mm, xt[:, lo:hi], start=True, stop=True
                )
                nc.vector.tensor_copy(a[:, CHUNK + lo : CHUNK + hi], gs[ch][:])
                if ch == 3:
                    # wrap tail: A cols [896, 1024) = chunk 3 cols [128, 256)
                    nc.vector.tensor_copy(a[:, 0:CHUNK], gs[ch][:, CHUNK:CW])

            # scatter: one contiguous 4.5 KiB region per row, row-split x2
            nc.sync.dma_start(gd_ap[0:64, :], a[0:64, :])
            nc.scalar.dma_start(gd_ap[64:CHUNK, :], a[64:CHUNK, :])

            # diagonal read: H[m, i] = gd_flat[128 + 1151 m + i], row-split x2
            for q in range(2):
                r0, r1 = q * 64, (q + 1) * 64
                diag = bass.AP(
                    gd, CHUNK + r0 * (AW - 1), [[AW - 1, 64], [1, DIM]]
                )
                eng = nc.sync if q == 0 else nc.scalar
                eng.dma_start(ht[r0:r1, :], diag)

            ot = sb.tile([1, DIM], f32)
            for ch in range(NCHUNKS):
                lo, hi = ch * CW, (ch + 1) * CW
                nc.tensor.matmul(
                    o[:, lo:hi], ones[:], ht[:, lo:hi], start=True, stop=True
                )
                nc.vector.tensor_copy(ot[:, lo:hi], o[:, lo:hi])
            nc.sync.dma_start(out.ap(), ot[:])

    nc.compile()
    _cached[key] = nc
    return nc


def _chop(x):
    if not USE_F32R:
        return x
    u = x.view(np.uint32) & np.uint32((0xFFFFFFFF << F32R_CHOP_BITS) & 0xFFFFFFFF)
    return u.view(np.float32)


def _in_maps(input1, input2):
    x1 = np.asarray(input1, dtype=np.float32)
    x2 = np.asarray(input2, dtype=np.float32)
    maps = []
    for c in range(NCORES):
        xin = np.empty((B, XW), np.float32)
        xin[:, 0:DIM] = x2
        xin[:, DIM:XW] = x1[:, c * CHUNK : (c + 1) * CHUNK]
        maps.append({"xin": _chop(np.ascontiguousarray(xin))})
    return maps


def _combine(results):
    total = np.zeros(DIM, np.float64)
    for c in range(NCORES):
        total += np.roll(results[c]["out"][0].astype(np.float64), CHUNK * c)
    return total.astype(np.float32).reshape(1, 1, DIM)


def _run(input1, input2, **kwargs):
    from concourse import bass_utils

    nc = _build()
    res = bass_utils.run_bass_kernel_spmd(
        nc, _in_maps(input1, input2), core_ids=list(range(NCORES)), **kwargs
    )
    return res


def kernel(input1, input2):
    res = _run(input1, input2)
    return _combine(res.results)


# revision 17
# speedup vs baseline: 1.3700x; 1.1598x over previous
"""Circular-convolution helper kernel for Trainium2 (8 NeuronCores).

Math: out[i] = sum_b sum_t x1[b,(i-t)%D] * x2[b,t]
            = sum_j G[j, (i-j)%D]   where G = x1^T @ x2  ([D, D], K=B contraction)

Sharding: G's rows are split across the 8 cores (core c owns rows
[128c, 128c+128)).  Per core:
  1. load xin = [x2 | x1c] as one [128, D+128] tensor (4.5 KiB rows keep the
     DMA descriptor count minimal), row-split across the two HWDGE queues
  2. A = x1c^T @ x2 into PSUM (K=128 fp32r matmul, 4 column chunks into
     separate PSUM banks)
  3. PSUM -> SBUF casts (fp32 -> fp32r) into a [128, 128+D] staging tile
     laid out as [A[:, 896:1024] | A] so the DRAM scatter is ONE contiguous
     region per row
  4. scatter to gd[128, 1152]: flat[1152 m + p] = staged row (4.5 KiB rows)
  5. diagonal read H[m, i] = A[m, (i-m) % D] = gd_flat[128 + 1151 m + i]
  6. ones-matmul partition collapse (fp32r): part[i] = sum_m H[m, i]
Host rotates each core's partial by 128c and sums.

Everything on the PE runs in fp16 (single-pass streaming, 10-bit mantissa
— same effective precision as tf32-style fp32r but half the DMA bytes).
PSUM accumulation stays fp32.
"""

import numpy as np

B = 128
DIM = 1024
NCORES = 8
CHUNK = DIM // NCORES  # 128 rows of G per core
NHALF = 512
NCHUNKS = 4
CW = DIM // NCHUNKS  # 256
XW = DIM + CHUNK  # packed input width: x2 | x1c
AW = CHUNK + DIM  # staging width: wrap tail | A

USE_FP16 = True


_cached = {}


def _build():
    key = ("nc", USE_FP16)
    if key in _cached:
        return _cached[key]

    import concourse.bass as bass
    import concourse.mybir as mybir
    from concourse import bacc
    from concourse.tile import TileContext

    f32 = mybir.dt.float32
    dt_mm = mybir.dt.float16 if USE_FP16 else f32

    nc = bacc.Bacc("TRN2", target_bir_lowering=False, debug=False)

    xin = nc.dram_tensor("xin", [B, XW], dt_mm, kind="ExternalInput")
    out = nc.dram_tensor("out", [1, DIM], f32, kind="ExternalOutput")
    gd = nc.dram_tensor("gd", [CHUNK, AW], dt_mm, kind="Internal")

    with TileContext(nc) as tc:
        with (
            tc.tile_pool(name="sb", bufs=1) as sb,
            tc.tile_pool(name="ps", bufs=1, space="PSUM") as ps,
        ):
            xt = sb.tile([B, XW], dt_mm)
            xin_ap = xin.ap()
            nc.sync.dma_start(xt[0:64, :], xin_ap[0:64, :])
            nc.scalar.dma_start(xt[64:B, :], xin_ap[64:B, :])
            x1_mm = xt[:, DIM:XW]

            gs = [
                ps.tile([CHUNK, CW], f32, name=f"g{i}", tag=f"g{i}")
                for i in range(NCHUNKS)
            ]
            a = sb.tile([CHUNK, AW], dt_mm)
            ht = sb.tile([CHUNK, DIM], dt_mm)
            ones = sb.tile([CHUNK, 1], dt_mm)
            nc.vector.memset(ones[:], 1.0)
            o = ps.tile([1, DIM], f32)
            gd_ap = gd.ap()

            # A chunks; staging layout: a[:, 0:128] = A[:, 896:1024] (wrap
            # tail), a[:, 128:1152] = A[:, 0:1024]
            order = [3, 0, 1, 2]
            for ch in order:
                lo, hi = ch * CW, (ch + 1) * CW
                nc.tensor.matmul(
                    gs[ch][:], x1_mm, xt[:, lo:hi], start=True, stop=True
                )
                nc.vector.tensor_copy(a[:, CHUNK + lo : CHUNK + hi], gs[ch][:])
                if ch == 3:
                    # wrap tail: A cols [896, 1024) = chunk 3 cols [128, 256)
                    nc.vector.tensor_copy(a[:, 0:CHUNK], gs[ch][:, CHUNK:CW])

            # scatter: one contiguous 4.5 KiB region per row, row-split x2
            nc.sync.dma_start(gd_ap[0:64, :], a[0:64, :])
            nc.scalar.dma_start(gd_ap[64:CHUNK, :], a[64:CHUNK, :])

            # diagonal read: H[m, i] = gd_flat[128 + 1151 m + i], row-split x2
            for q in range(2):
                r0, r1 = q * 64, (q + 1) * 64
                diag = bass.AP(
                    gd, CHUNK + r0 * (AW - 1), [[AW - 1, 64], [1, DIM]]
                )
                eng = nc.sync if q == 0 else nc.scalar
                eng.dma_start(ht[r0:r1, :], diag)

            ot = sb.tile([1, DIM], f32)
            for ch in range(NCHUNKS):
                lo, hi = ch * CW, (ch + 1) * CW
                nc.tensor.matmul(
                    o[:, lo:hi], ones[:], ht[:, lo:hi], start=True, stop=True
                )
                nc.vector.tensor_copy(ot[:, lo:hi], o[:, lo:hi])
            nc.sync.dma_start(out.ap(), ot[:])

    nc.compile()
    _cached[key] = nc
    return nc


def _in_maps(input1, input2):
    dt_in = np.float16 if USE_FP16 else np.float32
    x1 = np.asarray(input1, dtype=np.float32)
    x2 = np.asarray(input2, dtype=np.float32)
    maps = []
    for c in range(NCORES):
        xin = np.empty((B, XW), dt_in)
        xin[:, 0:DIM] = x2
        xin[:, DIM:XW] = x1[:, c * CHUNK : (c + 1) * CHUNK]
        maps.append({"xin": np.ascontiguousarray(xin)})
    return maps


def _combine(results):
    total = np.zeros(DIM, np.float64)
    for c in range(NCORES):
        total += np.roll(results[c]["out"][0].astype(np.float64), CHUNK * c)
    return total.astype(np.float32).reshape(1, 1, DIM)


def _run(input1, input2, **kwargs):
    from concourse import bass_utils

    nc = _build()
    res = bass_utils.run_bass_kernel_spmd(
        nc, _in_maps(input1, input2), core_ids=list(range(NCORES)), **kwargs
    )
    return res


def kernel(input1, input2):
    res = _run(input1, input2)
    return _combine(res.results)


# revision 19
# speedup vs baseline: 1.4299x; 1.0438x over previous
"""Circular-convolution helper kernel for Trainium2 (8 NeuronCores).

Math: out[i] = sum_b sum_t x1[b,(i-t)%D] * x2[b,t]
            = sum_j G[j, (i-j)%D]   where G = x1^T @ x2  ([D, D], K=B contraction)

Sharding: G's rows are split across the 8 cores (core c owns rows
[128c, 128c+128)).  Per core:
  1. load xin = [x2 | x1c] as one [128, D+128] tensor (4.5 KiB rows keep the
     DMA descriptor count minimal), row-split across the two HWDGE queues
  2. A = x1c^T @ x2 into PSUM (K=128 fp32r matmul, 4 column chunks into
     separate PSUM banks)
  3. PSUM -> SBUF casts (fp32 -> fp32r) into a [128, 128+D] staging tile
     laid out as [A[:, 896:1024] | A] so the DRAM scatter is ONE contiguous
     region per row
  4. scatter to gd[128, 1152]: flat[1152 m + p] = staged row (4.5 KiB rows)
  5. diagonal read H[m, i] = A[m, (i-m) % D] = gd_flat[128 + 1151 m + i]
  6. ones-matmul partition collapse (fp32r): part[i] = sum_m H[m, i]
Host rotates each core's partial by 128c and sums.

Everything on the PE runs in fp16 (single-pass streaming, 10-bit mantissa
— same effective precision as tf32-style fp32r but half the DMA bytes).
PSUM accumulation stays fp32.
"""

import numpy as np

B = 128
DIM = 1024
NCORES = 8
CHUNK = DIM // NCORES  # 128 rows of G per core
NHALF = 512
NCHUNKS = 4
CW = DIM // NCHUNKS  # 256
XW = DIM + CHUNK  # packed input width: x2 | x1c
AW = CHUNK + DIM  # staging width: wrap tail | A

USE_FP16 = True


_cached = {}


def _build():
    key = ("nc", USE_FP16)
    if key in _cached:
        return _cached[key]

    import concourse.bass as bass
    import concourse.mybir as mybir
    from concourse import bacc
    from concourse.tile import TileContext

    f32 = mybir.dt.float32
    dt_mm = mybir.dt.float16 if USE_FP16 else f32

    nc = bacc.Bacc("TRN2", target_bir_lowering=False, debug=False)

    xin = nc.dram_tensor("xin", [B, XW], dt_mm, kind="ExternalInput")
    out = nc.dram_tensor("out", [1, DIM], f32, kind="ExternalOutput")
    gd = nc.dram_tensor("gd", [CHUNK, AW], dt_mm, kind="Internal")

    with TileContext(nc) as tc:
        with (
            tc.tile_pool(name="sb", bufs=1) as sb,
            tc.tile_pool(name="ps", bufs=1, space="PSUM") as ps,
        ):
            xt = sb.tile([B, XW], dt_mm)
            xin_ap = xin.ap()
            nc.sync.dma_start(xt[0:64, :], xin_ap[0:64, :])
            nc.scalar.dma_start(xt[64:B, :], xin_ap[64:B, :])
            x1_mm = xt[:, DIM:XW]

            gs = [
                ps.tile([CHUNK, CW], f32, name=f"g{i}", tag=f"g{i}")
                for i in range(NCHUNKS)
            ]
            a = sb.tile([CHUNK, AW], dt_mm)
            ht = sb.tile([CHUNK, DIM], dt_mm)
            ones = sb.tile([CHUNK, 1], dt_mm)
            nc.vector.memset(ones[:], 1.0)
            os_ = [
                ps.tile([1, CW], f32, name=f"o{i}", tag=f"o{i}")
                for i in range(NCHUNKS)
            ]
            gd_ap = gd.ap()

            # A chunks; staging layout: a[:, 0:128] = A[:, 896:1024] (wrap
            # tail), a[:, 128:1152] = A[:, 0:1024]
            order = [3, 0, 1, 2]
            for i, ch in enumerate(order):
                lo, hi = ch * CW, (ch + 1) * CW
                nc.tensor.matmul(
                    gs[ch][:], x1_mm, xt[:, lo:hi], start=True, stop=True
                )
                # alternate cast engine so the cast chain is half as long
                if i % 2 == 0:
                    nc.scalar.copy(a[:, CHUNK + lo : CHUNK + hi], gs[ch][:])
                else:
                    nc.vector.tensor_copy(a[:, CHUNK + lo : CHUNK + hi], gs[ch][:])
                if ch == 3:
                    # wrap tail: A cols [896, 1024) = chunk 3 cols [128, 256)
                    nc.vector.tensor_copy(a[:, 0:CHUNK], gs[ch][:, CHUNK:CW])

            # scatter: one contiguous 4.5 KiB region per row, row-split x2
            nc.sync.dma_start(gd_ap[0:64, :], a[0:64, :])
            nc.scalar.dma_start(gd_ap[64:CHUNK, :], a[64:CHUNK, :])

            # diagonal read: H[m, i] = gd_flat[128 + 1151 m + i], row-split x2
            for q in range(2):
                r0, r1 = q * 64, (q + 1) * 64
                diag = bass.AP(
                    gd, CHUNK + r0 * (AW - 1), [[AW - 1, 64], [1, DIM]]
                )
                eng = nc.sync if q == 0 else nc.scalar
                eng.dma_start(ht[r0:r1, :], diag)

            # ones-matmul split over K (row halves) so the first half runs
            # as soon as the first diagonal read lands
            ot = sb.tile([1, DIM], f32)
            for ch in range(NCHUNKS):
                lo, hi = ch * CW, (ch + 1) * CW
                nc.tensor.matmul(
                    os_[ch][:], ones[0:64], ht[0:64, lo:hi],
                    start=True, stop=False,
                )
            for ch in range(NCHUNKS):
                lo, hi = ch * CW, (ch + 1) * CW
                nc.tensor.matmul(
                    os_[ch][:], ones[64:CHUNK], ht[64:CHUNK, lo:hi],
                    start=False, stop=True,
                )
                if ch % 2 == 0:
                    nc.scalar.copy(ot[:, lo:hi], os_[ch][:])
                else:
                    nc.vector.tensor_copy(ot[:, lo:hi], os_[ch][:])
            nc.sync.dma_start(out.ap(), ot[:])

    nc.compile()
    _cached[key] = nc
    return nc


def _in_maps(input1, input2):
    dt_in = np.float16 if USE_FP16 else np.float32
    x1 = np.asarray(input1, dtype=np.float32)
    x2 = np.asarray(input2, dtype=np.float32)
    maps = []
    for c in range(NCORES):
        xin = np.empty((B, XW), dt_in)
        xin[:, 0:DIM] = x2
        xin[:, DIM:XW] = x1[:, c * CHUNK : (c + 1) * CHUNK]
        maps.append({"xin": np.ascontiguousarray(xin)})
    return maps


def _combine(results):
    total = np.zeros(DIM, np.float64)
    for c in range(NCORES):
        total += np.roll(results[c]["out"][0].astype(np.float64), CHUNK * c)
    return total.astype(np.float32).reshape(1, 1, DIM)


def _run(input1, input2, **kwargs):
    from concourse import bass_utils

    nc = _build()
    res = bass_utils.run_bass_kernel_spmd(
        nc, _in_maps(input1, input2), core_ids=list(range(NCORES)), **kwargs
    )
    return res


def kernel(input1, input2):
    res = _run(input1, input2)
    return _combine(res.results)


# revision 21
# speedup vs baseline: 1.4558x; 1.0181x over previous
"""Circular-convolution helper kernel for Trainium2 (8 NeuronCores).

Math: out[i] = sum_b sum_t x1[b,(i-t)%D] * x2[b,t]
            = sum_j G[j, (i-j)%D]   where G = x1^T @ x2  ([D, D], K=B contraction)

Sharding: G's rows are split across the 8 cores (core c owns rows
[128c, 128c+128)).  Per core:
  1. load xin = [x1c | x2] as one [128, 128+D] tensor, split into a 2x2
     row/column grid across the two HWDGE queues so the first G chunk can
     start as soon as the first column block lands
  2. A = x1c^T @ x2 into PSUM (K=128 fp32r matmul, 4 column chunks into
     separate PSUM banks)
  3. PSUM -> SBUF casts (fp32 -> fp32r) into a [128, 128+D] staging tile
     laid out as [A[:, 896:1024] | A] so the DRAM scatter is ONE contiguous
     region per row
  4. scatter to gd[128, 1152]: flat[1152 m + p] = staged row (4.5 KiB rows)
  5. diagonal read H[m, i] = A[m, (i-m) % D] = gd_flat[128 + 1151 m + i]
  6. ones-matmul partition collapse (fp32r): part[i] = sum_m H[m, i]
Host rotates each core's partial by 128c and sums.

Everything on the PE runs in fp16 (single-pass streaming, 10-bit mantissa
— same effective precision as tf32-style fp32r but half the DMA bytes).
PSUM accumulation stays fp32.
"""

import numpy as np

B = 128
DIM = 1024
NCORES = 8
CHUNK = DIM // NCORES  # 128 rows of G per core
NHALF = 512
NCHUNKS = 4
CW = DIM // NCHUNKS  # 256
XW = DIM + CHUNK  # packed input width: x1c | x2
XSPLIT = CHUNK + NHALF  # first column block: x1c + x2[:, 0:512]
AW = CHUNK + DIM  # staging width: wrap tail | A

USE_FP16 = True


_cached = {}


def _build():
    key = ("nc", USE_FP16)
    if key in _cached:
        return _cached[key]

    import concourse.bass as bass
    import concourse.mybir as mybir
    from concourse import bacc
    from concourse.tile import TileContext

    f32 = mybir.dt.float32
    dt_mm = mybir.dt.float16 if USE_FP16 else f32

    nc = bacc.Bacc("TRN2", target_bir_lowering=False, debug=False)

    xin = nc.dram_tensor("xin", [B, XW], dt_mm, kind="ExternalInput")
    out = nc.dram_tensor("out", [1, DIM], f32, kind="ExternalOutput")
    gd = nc.dram_tensor("gd", [CHUNK, AW], dt_mm, kind="Internal")

    with TileContext(nc) as tc:
        with (
            tc.tile_pool(name="sb", bufs=1) as sb,
            tc.tile_pool(name="ps", bufs=1, space="PSUM") as ps,
        ):
            xt = sb.tile([B, XW], dt_mm)
            xin_ap = xin.ap()
            nc.sync.dma_start(xt[0:64, 0:XSPLIT], xin_ap[0:64, 0:XSPLIT])
            nc.scalar.dma_start(xt[64:B, 0:XSPLIT], xin_ap[64:B, 0:XSPLIT])
            nc.sync.dma_start(xt[0:64, XSPLIT:XW], xin_ap[0:64, XSPLIT:XW])
            nc.scalar.dma_start(xt[64:B, XSPLIT:XW], xin_ap[64:B, XSPLIT:XW])
            x1_mm = xt[:, 0:CHUNK]

            gs = [
                ps.tile([CHUNK, CW], f32, name=f"g{i}", tag=f"g{i}")
                for i in range(NCHUNKS)
            ]
            a = sb.tile([CHUNK, AW], dt_mm)
            ht = sb.tile([CHUNK, DIM], dt_mm)
            ones = sb.tile([CHUNK, 1], dt_mm)
            nc.vector.memset(ones[:], 1.0)
            os_ = [
                ps.tile([1, CW], f32, name=f"o{i}", tag=f"o{i}")
                for i in range(NCHUNKS)
            ]
            gd_ap = gd.ap()

            # A chunks; staging layout: a[:, 0:128] = A[:, 896:1024] (wrap
            # tail), a[:, 128:1152] = A[:, 0:1024]
            order = [0, 1, 2, 3]
            for i, ch in enumerate(order):
                lo, hi = ch * CW, (ch + 1) * CW
                nc.tensor.matmul(
                    gs[ch][:], x1_mm, xt[:, CHUNK + lo : CHUNK + hi],
                    start=True, stop=True,
                )
                # alternate cast engine so the cast chain is half as long
                if i % 2 == 0:
                    nc.scalar.copy(a[:, CHUNK + lo : CHUNK + hi], gs[ch][:])
                else:
                    nc.vector.tensor_copy(a[:, CHUNK + lo : CHUNK + hi], gs[ch][:])
                if ch == 3:
                    # wrap tail: A cols [896, 1024) = chunk 3 cols [128, 256)
                    nc.vector.tensor_copy(a[:, 0:CHUNK], gs[ch][:, CHUNK:CW])

            # scatter + diagonal read, chained in 4 row blocks so reads
            # stream right behind the writes.
            # H[m, i] = gd_flat[128 + 1151 m + i]
            for q in range(4):
                r0, r1 = q * 32, (q + 1) * 32
                w_eng = nc.sync if q % 2 == 0 else nc.scalar
                w_eng.dma_start(gd_ap[r0:r1, :], a[r0:r1, :])
            for q in range(4):
                r0, r1 = q * 32, (q + 1) * 32
                diag = bass.AP(
                    gd, CHUNK + r0 * (AW - 1), [[AW - 1, 32], [1, DIM]]
                )
                r_eng = nc.sync if q % 2 == 0 else nc.scalar
                r_eng.dma_start(ht[r0:r1, :], diag)

            # ones-matmul split over K (row halves) so the first half runs
            # as soon as the first diagonal read lands
            ot = sb.tile([1, DIM], f32)
            for ch in range(NCHUNKS):
                lo, hi = ch * CW, (ch + 1) * CW
                nc.tensor.matmul(
                    os_[ch][:], ones[0:64], ht[0:64, lo:hi],
                    start=True, stop=False,
                )
            for ch in range(NCHUNKS):
                lo, hi = ch * CW, (ch + 1) * CW
                nc.tensor.matmul(
                    os_[ch][:], ones[64:CHUNK], ht[64:CHUNK, lo:hi],
                    start=False, stop=True,
                )
                if ch % 2 == 0:
                    nc.scalar.copy(ot[:, lo:hi], os_[ch][:])
                else:
                    nc.vector.tensor_copy(ot[:, lo:hi], os_[ch][:])
            nc.sync.dma_start(out.ap(), ot[:])

    nc.compile()
    _cached[key] = nc
    return nc


def _in_maps(input1, input2):
    dt_in = np.float16 if USE_FP16 else np.float32
    x1 = np.asarray(input1, dtype=np.float32)
    x2 = np.asarray(input2, dtype=np.float32)
    maps = []
    for c in range(NCORES):
        xin = np.empty((B, XW), dt_in)
        xin[:, 0:CHUNK] = x1[:, c * CHUNK : (c + 1) * CHUNK]
        xin[:, CHUNK:XW] = x2
        maps.append({"xin": np.ascontiguousarray(xin)})
    return maps


def _combine(results):
    total = np.zeros(DIM, np.float64)
    for c in range(NCORES):
        total += np.roll(results[c]["out"][0].astype(np.float64), CHUNK * c)
    return total.astype(np.float32).reshape(1, 1, DIM)


def _run(input1, input2, **kwargs):
    from concourse import bass_utils

    nc = _build()
    res = bass_utils.run_bass_kernel_spmd(
        nc, _in_maps(input1, input2), core_ids=list(range(NCORES)), **kwargs
    )
    return res


def kernel(input1, input2):
    res = _run(input1, input2)
    return _combine(res.results)
